# revision 3
# baseline (speedup 1.0000x reference)
"""Trainium2 Bass kernel v3 (promoted) for nn_Net_9560597201379 (SNN encoder/decoder MLP).

Per-core algebra (rows_e = 128 encoder rows (t,b); decoder rows split into two
halves h∈{A,B} of 512 rows (se-groups 0-3 / 4-7)):

  cb1 = x @ W1^T + b1                       (PSUM fp32, 8 mm + bias mm)
  P1_t = gam_t*cb1 - th_t                   (8 ACT precomputes)
  enc t=1..8:  bin1 = (P1_t > S1) ; S1 += th_{t+1}*bin1
               psn2 += bin1 @ (th_{t+1}*0.9*W2^T)   [device-scaled copies]
               spk01[t-1] = ((psn2 - thr2_t) > S2) ; S2 += th_{t+1}*spk01
  cur3 (half): psc3[mc] = W3^T @ spk01[se-half]     (1 mm per mc, f32r ap512)
  dec t=1..8 (half): t=1: sneg = (cb3 > thr31s)*(-th_2)   [from PSUM]
               t>=2: n3 = gam_t*cb3 + S3n ; sneg = (n3 > thr3_t)*(-th_{t+1})
               t<8:  S3n += sneg
               ps4 += sneg @ (-0.9*W4^T)            [f32r, 8 mm ap512]
               evac ps4 -> fp16 SBUF -> DRAM        [per half-step]
  HOST: mem[t] = N4_t * 0.9^t + (0.9^t*gam_t)*b4 ; spk out == 0 exactly.

Half A starts after encoder step 4 to overlap decoder with encoder and start
output DMA early. PSUM: psc1+psn2 (enc) | psc3h (transient) | ps4A/ps4B 4+4.
"""

import os
import sys

import numpy as np

sys.path.insert(0, "/opt/trn_rl_repo")
sys.path.insert(0, "/opt/trn_rl_repo/concourse")

import concourse.bass as bass  # noqa: E402
import concourse.mybir as mybir  # noqa: E402
from concourse import bacc  # noqa: E402
from concourse import tile  # noqa: E402
from concourse.bass_utils import run_bass_kernel_spmd  # noqa: E402

F32 = mybir.dt.float32
F32R = mybir.dt.float32r
F16 = mybir.dt.float16
AL = mybir.AluOpType
AF = mybir.ActivationFunctionType

T = 8
B = 128
NCORES = 8
BS = B // NCORES          # 16 batch rows per core
F_IN = 512
H1 = 256
H2 = 128
H3 = 256
F4 = 512
ROWS_E = T * BS           # 128 encoder rows (t, b)
ROWS_H = 4 * ROWS_E       # 512 decoder rows per half (se, t, b)
BETA = 0.9

TH = [np.float32(BETA ** (-t)) for t in range(0, 11)]
BPOW = [np.float32(BETA ** t) for t in range(0, 11)]
GAM = [np.float32(0.0)]
for _t in range(1, 11):
    GAM.append(np.float32(GAM[_t - 1] + TH[_t]))


def build_module():
    nc = bacc.Bacc(
        "TRN2",
        target_bir_lowering=False,
        debug=False,
        enable_asserts=False,
    )

    # ---- inputs (host-prepped layouts; per-partition contiguous) ----
    xt_d = nc.dram_tensor("xt", [128, 4, ROWS_E], F32, kind="ExternalInput")
    w1t_d = nc.dram_tensor("w1t", [128, 4, H1], F32, kind="ExternalInput")
    b1_d = nc.dram_tensor("b1r", [1, H1], F32, kind="ExternalInput")
    w2t_d = nc.dram_tensor("w2t", [128, 2, H2], F32, kind="ExternalInput")
    w3t_d = nc.dram_tensor("w3t", [128, 2, H2], F32, kind="ExternalInput")
    w4tn_d = nc.dram_tensor("w4tn", [128, 2, F4], F32, kind="ExternalInput")
    # thr pack: [:, 0:8] thr2_t ; [:, 8:16] thr3_t mc0 ; [:, 16:24] thr3_t mc1
    #           [:, 24] thr31s mc0 ; [:, 25] thr31s mc1
    thr_d = nc.dram_tensor("thrp", [128, 26], F32, kind="ExternalInput")
    out16_d = nc.dram_tensor("out16", [T, 8 * 128, F4], F16, kind="ExternalOutput")

    with tile.TileContext(nc) as tc:
        with (
            tc.tile_pool(name="const", bufs=1) as cp,
            tc.tile_pool(name="state", bufs=1) as sp,
            tc.tile_pool(name="m4p", bufs=4) as m4p,
        ):
            # ---------------- input DMAs (spread across queues) ----------
            ones_row = cp.tile([1, 512], F32, name="ones_row")
            nc.vector.memset(ones_row[:], 1.0)

            xt = cp.tile([128, 4, ROWS_E], F32, name="xt")
            nc.sync.dma_start(out=xt[:], in_=xt_d.ap())
            w1t = cp.tile([128, 4, H1], F32, name="w1t")
            nc.sync.dma_start(out=w1t[:], in_=w1t_d.ap())
            b1_sb = cp.tile([1, H1], F32, name="b1_sb")
            nc.scalar.dma_start(out=b1_sb[:], in_=b1_d.ap())
            thrp = cp.tile([128, 26], F32, name="thrp")
            w2t = cp.tile([128, 2, H2], F32, name="w2t")
            w3t = cp.tile([128, 2, H2], F32, name="w3t")
            w4tn = cp.tile([128, 2, F4], F32R, name="w4tn")

            # ---------------- state tiles ----------------
            cb1 = sp.tile([128, 2, ROWS_E], F32, name="cb1")
            P1 = sp.tile([128, 8, 2, ROWS_E], F32, name="P1")
            S1 = sp.tile([128, 2, ROWS_E], F32, name="S1")
            nc.vector.memset(S1[:], 0.0)
            bin1 = sp.tile([128, 8, 2, ROWS_E], F32, name="bin1")
            spk01 = sp.tile([128, 8, ROWS_E], F32, name="spk01")
            S2 = sp.tile([128, ROWS_E], F32, name="S2")
            nc.gpsimd.memset(S2[:], 0.0)
            w2t8 = sp.tile([128, 8, 2, H2], F32, name="w2t8")

            cb3 = [[sp.tile([128, ROWS_H], F32, name=f"cb3_{h}{m}")
                    for m in range(2)] for h in range(2)]
            S3n = [[sp.tile([128, ROWS_H], F32, name=f"S3n_{h}{m}")
                    for m in range(2)] for h in range(2)]
            n3 = [[sp.tile([128, ROWS_H], F32, name=f"n3_{h}{m}")
                   for m in range(2)] for h in range(2)]
            sneg = [[[None] * 2 for _ in range(8)] for _ in range(2)]


            # ---------------- PE warmup (p-state ramp) ----------------
            psW_cm = tc.tile_pool(name="psW", bufs=1, space="PSUM")
            psW = psW_cm.__enter__()
            wtile = psW.tile([128, F4], F32, name="warm")
            for _ in range(5):
                nc.tensor.matmul(
                    wtile[:, 0:128],
                    lhsT=ones_row[0:1, 0:128],
                    rhs=ones_row[0:1, 0:128],
                    start=True, stop=True, skip_group_check=True,
                )
            psW_cm.__exit__(None, None, None)

            # ---------------- PSUM pools ----------------
            # stack order: psc3AB (encoder-long) -> psA -> (both closed)
            # -> ps4A -> ps4B
            ps4 = [None, None]
            ps4_cm = [None, None]
            psc3 = [None, None]
            psc3A_cm = tc.tile_pool(name="psc3A", bufs=1, space="PSUM",
                                    side="left")
            psc3[0] = psc3A_cm.__enter__().tile(
                [128, 2, ROWS_H], F32, name="psc3_0")
            psc3B_cm = tc.tile_pool(name="psc3B", bufs=1, space="PSUM",
                                    side="right")
            psc3[1] = psc3B_cm.__enter__().tile(
                [128, 2, ROWS_H], F32, name="psc3_1")

            psA_cm = tc.tile_pool(name="psA", bufs=1, space="PSUM")
            psA = psA_cm.__enter__()
            psc1 = psA.tile([128, 2, ROWS_E], F32, name="psc1")
            psn2 = psA.tile([128, ROWS_E], F32, name="psn2")

            # ---------------- cur1 ----------------
            for fc in range(2):
                for kc in range(4):
                    nc.tensor.matmul(
                        psc1[:, fc, :],
                        lhsT=w1t[:, kc, fc * 128:(fc + 1) * 128],
                        rhs=xt[:, kc, :],
                        start=(kc == 0),
                        stop=False,
                        skip_group_check=True,
                    )
                nc.tensor.matmul(
                    psc1[:, fc, :],
                    lhsT=b1_sb[0:1, fc * 128:(fc + 1) * 128],
                    rhs=ones_row[0:1, 0:ROWS_E],
                    start=False,
                    stop=(fc == 1),
                    skip_group_check=True,
                )
            nc.scalar.dma_start(out=thrp[:], in_=thr_d.ap())
            nc.scalar.dma_start(out=w2t[:], in_=w2t_d.ap())
            nc.scalar.activation(cb1[:], psc1[:], AF.Copy)

            # P1_t = gam_t*cb1 - th_t (ACT), w2t8_t = th_{t+1}*w2t (ACT)
            nc.scalar.activation(
                w2t8[:, 0], w2t[:], AF.Copy, scale=float(TH[2]),
            )

            def gen_p1(t):
                nc.scalar.activation(
                    P1[:, t - 1], cb1[:], AF.Copy,
                    bias=float(-TH[t]), scale=float(GAM[t]),
                )
                nc.scalar.activation(
                    w2t8[:, t - 1], w2t[:], AF.Copy,
                    scale=float(TH[t + 1]),
                )

            for t in range(2, 9):
                gen_p1(t)

            # ---------------- encoder scan + staggered decoder ----------
            def enc_step(t):
                b = bin1[:, t - 1]
                if t == 1:
                    # S1 == 0: bin = (gam_1*cb1 - th_1 > 0) <=> psc1 > 1
                    nc.vector.tensor_scalar(
                        b[:], psc1[:], 1.0, None, AL.is_gt,
                    )
                else:
                    nc.vector.tensor_tensor(
                        out=b[:], in0=P1[:, t - 1], in1=S1[:],
                        op=AL.is_gt,
                    )
                if t < 8:
                    nc.vector.scalar_tensor_tensor(
                        out=S1[:], in0=b[:], scalar=float(TH[t + 1]),
                        in1=S1[:], op0=AL.mult, op1=AL.add,
                    )
                for kc in range(2):
                    nc.tensor.matmul(
                        psn2[:],
                        lhsT=w2t8[:, t - 1, kc, :],
                        rhs=b[:, kc, :],
                        start=(t == 1 and kc == 0),
                        stop=(t == 8 and kc == 1),
                        skip_group_check=True,
                    )
                nc.vector.scalar_tensor_tensor(
                    out=spk01[:, t - 1, :], in0=psn2[:],
                    scalar=thrp[:, t - 1:t], in1=S2[:],
                    op0=AL.subtract, op1=AL.is_gt,
                )
                if t < 8:
                    nc.vector.scalar_tensor_tensor(
                        out=S2[:], in0=spk01[:, t - 1, :],
                        scalar=float(TH[t + 1]), in1=S2[:],
                        op0=AL.mult, op1=AL.add,
                    )
                # incremental cur3: column slice for se = t-1 (fills PE gaps)
                h, sl = (t - 1) // 4, (t - 1) % 4
                for mc in range(2):
                    nc.tensor.matmul(
                        psc3[h][:, mc, sl * 128:(sl + 1) * 128],
                        lhsT=w3t[:, mc, :],
                        rhs=spk01[:, t - 1, :],
                        start=True,
                        stop=True,
                        skip_group_check=True,
                    )

            def half_ramp(h):
                """step-1 spikes + cb3 evac from psc3[h] (already filled)."""
                nc.scalar.activation(cb3[h][0][:], psc3[h][:, 0, :], AF.Copy)
                nc.scalar.activation(cb3[h][1][:], psc3[h][:, 1, :], AF.Copy)
                for mc in range(2):
                    s = sp.tile([128, ROWS_H], F32R, name=f"sneg_{h}_0_{mc}")
                    sneg[h][0][mc] = s
                    nc.vector.tensor_scalar(
                        s[:], psc3[h][:, mc, :],
                        thrp[:, 24 + mc:25 + mc], float(-TH[2]),
                        AL.is_gt, AL.mult,
                    )

            def dec_step(h, t):
                """One decoder step for half h: spikes (t>=2), mm, evac, DMA."""
                if t >= 2:
                    for mc in range(2):
                        s = sp.tile([128, ROWS_H], F32R, name=f"sneg_{h}_{t - 1}_{mc}")
                        sneg[h][t - 1][mc] = s
                        nc.vector.scalar_tensor_tensor(
                            out=n3[h][mc][:], in0=cb3[h][mc][:],
                            scalar=float(GAM[t]), in1=S3n[h][mc][:],
                            op0=AL.mult, op1=AL.add,
                        )
                        eng = nc.vector if mc == 0 else nc.gpsimd
                        eng.tensor_scalar(
                            s[:], n3[h][mc][:],
                            thrp[:, 8 + 8 * mc + t - 1:9 + 8 * mc + t - 1],
                            float(-TH[t + 1]), AL.is_gt, AL.mult,
                        )
                s = sneg[h][t - 1]
                for kc in range(2):
                    for rc in range(4):
                        nc.tensor.matmul(
                            ps4[h][:, rc, :],
                            lhsT=s[kc][:, rc * 128:(rc + 1) * 128],
                            rhs=w4tn[:, kc, :],
                            start=(t == 1 and kc == 0),
                            stop=(t == 8 and kc == 1),
                            skip_group_check=True,
                        )
                # evacuate this step's snapshot to fp16 and DMA out (ACT only)
                m4 = m4p.tile([128, 4, F4], F16, name=f"m4_{h}")
                dview = out16_d.ap()[t - 1].rearrange("(s p) f -> p s f", p=128)
                if t == 8:
                    # final step: DVE is drained, split evac ACT || DVE
                    nc.scalar.activation(m4[:, 0:3, :], ps4[h][:, 0:3, :], AF.Copy)
                    nc.vector.tensor_copy(out=m4[:, 3:4, :], in_=ps4[h][:, 3:4, :])
                    nc.sync.dma_start(
                        out=dview[:, 4 * h:4 * h + 3, :], in_=m4[:, 0:3, :])
                    nc.sync.dma_start(
                        out=dview[:, 4 * h + 3:4 * h + 4, :], in_=m4[:, 3:4, :])
                else:
                    nc.scalar.activation(m4[:], ps4[h][:], AF.Copy)
                    nc.sync.dma_start(
                        out=dview[:, 4 * h:4 * h + 4, :], in_=m4[:],
                    )
                # spike-history update (off the critical path)
                if t == 1:
                    nc.vector.tensor_copy(
                        out=S3n[h][0][:], in_=sneg[h][0][0][:].bitcast(F32))
                    nc.gpsimd.tensor_copy(
                        out=S3n[h][1][:], in_=sneg[h][0][1][:].bitcast(F32))
                elif t < 8:
                    for mc in range(2):
                        eng = nc.vector if mc == 0 else nc.gpsimd
                        eng.tensor_tensor(
                            out=S3n[h][mc][:], in0=S3n[h][mc][:],
                            in1=sneg[h][t - 1][mc][:].bitcast(F32), op=AL.add,
                        )

            # encoder steps 1-8 (decoder-free)
            enc_step(1)
            nc.scalar.dma_start(out=w3t[:], in_=w3t_d.ap())
            nc.gpsimd.dma_start(out=w4tn[:], in_=w4tn_d.ap())
            for t in range(2, 9):
                enc_step(t)
            psA_cm.__exit__(None, None, None)
            # both ramps up front: ACT copies first, then DVE spikes
            for h in range(2):
                nc.scalar.activation(cb3[h][0][:], psc3[h][:, 0, :], AF.Copy)
                nc.scalar.activation(cb3[h][1][:], psc3[h][:, 1, :], AF.Copy)
            for h in range(2):
                for mc in range(2):
                    s = sp.tile([128, ROWS_H], F32R, name=f"sneg_{h}_0_{mc}")
                    sneg[h][0][mc] = s
                    nc.vector.tensor_scalar(
                        s[:], psc3[h][:, mc, :],
                        thrp[:, 24 + mc:25 + mc], float(-TH[2]),
                        AL.is_gt, AL.mult,
                    )
            psc3A_cm.__exit__(None, None, None)
            ps4_cm[0] = tc.tile_pool(name="ps4A", bufs=1, space="PSUM",
                                     side="left")
            ps4[0] = ps4_cm[0].__enter__().tile(
                [128, 4, F4], F32, name="ps4_0")
            psc3B_cm.__exit__(None, None, None)
            ps4_cm[1] = tc.tile_pool(name="ps4B", bufs=1, space="PSUM",
                                     side="right")
            ps4[1] = ps4_cm[1].__enter__().tile(
                [128, 4, F4], F32, name="ps4_1")
            for t in range(1, 9):
                dec_step(0, t)
                dec_step(1, t)
            ps4_cm[1].__exit__(None, None, None)
            ps4_cm[0].__exit__(None, None, None)

    nc.compile()
    return nc


_NC_CACHE = None


def _get_module():
    global _NC_CACHE
    if _NC_CACHE is None:
        _NC_CACHE = build_module()
    return _NC_CACHE


def _prep_shared(W1, b1, W2, b2, W3, b3, W4):
    f = np.float32
    w1t = np.ascontiguousarray(W1.T.reshape(4, 128, H1).transpose(1, 0, 2), f)
    b1r = np.ascontiguousarray(b1.reshape(1, H1), f)
    w2t = np.ascontiguousarray(
        (BETA * W2.T).astype(f).reshape(2, 128, H2).transpose(1, 0, 2), f)
    w3t = np.ascontiguousarray(
        W3.T.reshape(128, 2, H2), f)  # [in128, mc, out128]: W3.T is [128, 256]
    w4tn = np.ascontiguousarray(
        (-BETA * W4.T).astype(f).reshape(2, 128, F4).transpose(1, 0, 2), f)

    thrp = np.zeros((128, 26), f)
    for t in range(1, 9):
        thrp[:, t - 1] = TH[t] - GAM[t] * b2
    thr3 = np.stack([(TH[t] - GAM[t] * b3).astype(f) for t in range(1, 9)])  # [8,256]
    for mc in range(2):
        thrp[:, 8 + 8 * mc:16 + 8 * mc] = thr3[:, mc * 128:(mc + 1) * 128].T
        thrp[:, 24 + mc] = (BETA * thr3[0, mc * 128:(mc + 1) * 128]).astype(f)
    return dict(
        w1t=w1t, b1r=b1r, w2t=w2t, w3t=w3t, w4tn=w4tn,
        thrp=np.ascontiguousarray(thrp),
    )


def kernel(x, W1, b1, W2, b2, W3, b3, W4, b4):
    f = np.float32
    x = np.asarray(x, f)
    shared = _prep_shared(
        np.asarray(W1, f), np.asarray(b1, f), np.asarray(W2, f),
        np.asarray(b2, f), np.asarray(W3, f), np.asarray(b3, f),
        np.asarray(W4, f))
    b4 = np.asarray(b4, f)

    nc = _get_module()
    in_maps = []
    for i in range(NCORES):
        m = dict(shared)
        xs = x[:, i * BS:(i + 1) * BS, :].reshape(ROWS_E, F_IN)  # rows (t,b)
        m["xt"] = np.ascontiguousarray(
            xs.T.reshape(4, 128, ROWS_E).transpose(1, 0, 2), f)
        in_maps.append(m)

    trace = os.environ.get("KERNEL_TRACE", "0") == "1"
    res = run_bass_kernel_spmd(
        nc, in_maps, core_ids=list(range(NCORES)), trace=trace
    )
    if trace and res.exec_time_ns is not None:
        print(f"HW exec time: {res.exec_time_ns} ns")

    # host reconstruction: mem[t] = N4_t * beta^t + (beta^t*gam_t) * b4
    scale = np.array([BPOW[t] for t in range(1, 9)], f)
    bias = scale[:, None] * np.array([GAM[t] for t in range(1, 9)], f)[:, None] \
        * b4[None, :]
    mem = np.empty((T, T, T, B, F4), dtype=f)
    for i in range(NCORES):
        n4 = res.results[i]["out16"].astype(f)  # [8, 1024(se,t,b), 512]
        m4 = n4 * scale[:, None, None] + bias[:, None, :]
        mem[:, :, :, i * BS:(i + 1) * BS, :] = m4.reshape(T, T, T, BS, F4)
    spk = np.zeros((T, T, T, B, F4), dtype=f)
    return mem, spk


# revision 4
# speedup vs baseline: 1.0220x; 1.0220x over previous
"""Trainium2 Bass kernel v3 (promoted) for nn_Net_9560597201379 (SNN encoder/decoder MLP).

Per-core algebra (rows_e = 128 encoder rows (t,b); decoder rows split into two
halves h∈{A,B} of 512 rows (se-groups 0-3 / 4-7)):

  cb1 = x @ W1^T + b1                       (PSUM fp32, 8 mm + bias mm)
  P1_t = gam_t*cb1 - th_t                   (8 ACT precomputes)
  enc t=1..8:  bin1 = (P1_t > S1) ; S1 += th_{t+1}*bin1
               psn2 += bin1 @ (th_{t+1}*0.9*W2^T)   [device-scaled copies]
               spk01[t-1] = ((psn2 - thr2_t) > S2) ; S2 += th_{t+1}*spk01
  cur3 (half): psc3[mc] = W3^T @ spk01[se-half]     (1 mm per mc, f32r ap512)
  dec t=1..8 (half): t=1: sneg = (cb3 > thr31s)*(-th_2)   [from PSUM]
               t>=2: n3 = gam_t*cb3 + S3n ; sneg = (n3 > thr3_t)*(-th_{t+1})
               t<8:  S3n += sneg
               ps4 += sneg @ (-0.9*W4^T)            [f32r, 8 mm ap512]
               evac ps4 -> fp16 SBUF -> DRAM        [per half-step]
  HOST: mem[t] = N4_t * 0.9^t + (0.9^t*gam_t)*b4 ; spk out == 0 exactly.

Half A starts after encoder step 4 to overlap decoder with encoder and start
output DMA early. PSUM: psc1+psn2 (enc) | psc3h (transient) | ps4A/ps4B 4+4.
"""

import os
import sys

import numpy as np

sys.path.insert(0, "/opt/trn_rl_repo")
sys.path.insert(0, "/opt/trn_rl_repo/concourse")

import concourse.bass as bass  # noqa: E402
import concourse.mybir as mybir  # noqa: E402
from concourse import bacc  # noqa: E402
from concourse import tile  # noqa: E402
from concourse.bass_utils import run_bass_kernel_spmd  # noqa: E402

F32 = mybir.dt.float32
F32R = mybir.dt.float32r
F16 = mybir.dt.float16
AL = mybir.AluOpType
AF = mybir.ActivationFunctionType

T = 8
B = 128
NCORES = 8
BS = B // NCORES          # 16 batch rows per core
F_IN = 512
H1 = 256
H2 = 128
H3 = 256
F4 = 512
ROWS_E = T * BS           # 128 encoder rows (t, b)
ROWS_H = 4 * ROWS_E       # 512 decoder rows per half (se, t, b)
BETA = 0.9

TH = [np.float32(BETA ** (-t)) for t in range(0, 11)]
BPOW = [np.float32(BETA ** t) for t in range(0, 11)]
GAM = [np.float32(0.0)]
for _t in range(1, 11):
    GAM.append(np.float32(GAM[_t - 1] + TH[_t]))


def build_module():
    nc = bacc.Bacc(
        "TRN2",
        target_bir_lowering=False,
        debug=False,
        enable_asserts=False,
    )

    # ---- inputs (host-prepped layouts; per-partition contiguous) ----
    xt_d = nc.dram_tensor("xt", [128, 4, ROWS_E], F32, kind="ExternalInput")
    w1t_d = nc.dram_tensor("w1t", [128, 4, H1], F32, kind="ExternalInput")
    b1_d = nc.dram_tensor("b1r", [1, H1], F32, kind="ExternalInput")
    w2t_d = nc.dram_tensor("w2t", [128, 2, H2], F32, kind="ExternalInput")
    w3t_d = nc.dram_tensor("w3t", [128, 2, H2], F32, kind="ExternalInput")
    w4tn_d = nc.dram_tensor("w4tn", [128, 2, F4], F32, kind="ExternalInput")
    # thr pack: [:, 0:8] thr2_t ; [:, 8:16] thr3_t mc0 ; [:, 16:24] thr3_t mc1
    #           [:, 24] thr31s mc0 ; [:, 25] thr31s mc1
    thr_d = nc.dram_tensor("thrp", [128, 26], F32, kind="ExternalInput")
    out16_d = nc.dram_tensor("out16", [T, 8 * 128, F4], F16, kind="ExternalOutput")

    with tile.TileContext(nc) as tc:
        with (
            tc.tile_pool(name="const", bufs=1) as cp,
            tc.tile_pool(name="state", bufs=1) as sp,
            tc.tile_pool(name="m4p", bufs=4) as m4p,
        ):
            # ---------------- input DMAs (spread across queues) ----------
            ones_row = cp.tile([1, 512], F32, name="ones_row")
            nc.vector.memset(ones_row[:], 1.0)

            xt = cp.tile([128, 4, ROWS_E], F32, name="xt")
            nc.sync.dma_start(out=xt[:], in_=xt_d.ap())
            w1t = cp.tile([128, 4, H1], F32, name="w1t")
            nc.sync.dma_start(out=w1t[:], in_=w1t_d.ap())
            b1_sb = cp.tile([1, H1], F32, name="b1_sb")
            nc.scalar.dma_start(out=b1_sb[:], in_=b1_d.ap())
            thrp = cp.tile([128, 26], F32, name="thrp")
            w2t = cp.tile([128, 2, H2], F32, name="w2t")
            w3t = cp.tile([128, 2, H2], F32, name="w3t")
            w4tn = cp.tile([128, 2, F4], F32R, name="w4tn")

            # ---------------- state tiles ----------------
            cb1 = sp.tile([128, 2, ROWS_E], F32, name="cb1")
            P1 = sp.tile([128, 8, 2, ROWS_E], F32, name="P1")
            S1 = sp.tile([128, 2, ROWS_E], F32, name="S1")
            nc.vector.memset(S1[:], 0.0)
            bin1 = sp.tile([128, 8, 2, ROWS_E], F32, name="bin1")
            spk01 = sp.tile([128, 8, ROWS_E], F32, name="spk01")
            S2 = sp.tile([128, ROWS_E], F32, name="S2")
            nc.gpsimd.memset(S2[:], 0.0)
            w2t8 = sp.tile([128, 8, 2, H2], F32, name="w2t8")

            cb3 = [[sp.tile([128, ROWS_H], F32, name=f"cb3_{h}{m}")
                    for m in range(2)] for h in range(2)]
            S3n = [[sp.tile([128, ROWS_H], F32, name=f"S3n_{h}{m}")
                    for m in range(2)] for h in range(2)]
            n3 = [[sp.tile([128, ROWS_H], F32, name=f"n3_{h}{m}")
                   for m in range(2)] for h in range(2)]
            sneg = [[[None] * 2 for _ in range(8)] for _ in range(2)]


            # ---------------- PE warmup (p-state ramp) ----------------
            psW_cm = tc.tile_pool(name="psW", bufs=1, space="PSUM")
            psW = psW_cm.__enter__()
            wtile = psW.tile([128, F4], F32, name="warm")
            for _ in range(5):
                nc.tensor.matmul(
                    wtile[:, 0:128],
                    lhsT=ones_row[0:1, 0:128],
                    rhs=ones_row[0:1, 0:128],
                    start=True, stop=True, skip_group_check=True,
                )
            psW_cm.__exit__(None, None, None)

            # ---------------- PSUM pools ----------------
            # stack order: psc3AB (encoder-long) -> psA -> (both closed)
            # -> ps4A -> ps4B
            ps4 = [None, None]
            ps4_cm = [None, None]
            psc3 = [None, None]
            psc3A_cm = tc.tile_pool(name="psc3A", bufs=1, space="PSUM",
                                    side="left")
            psc3[0] = psc3A_cm.__enter__().tile(
                [128, 2, ROWS_H], F32, name="psc3_0")
            psc3B_cm = tc.tile_pool(name="psc3B", bufs=1, space="PSUM",
                                    side="right")
            psc3[1] = psc3B_cm.__enter__().tile(
                [128, 2, ROWS_H], F32, name="psc3_1")

            psA_cm = tc.tile_pool(name="psA", bufs=1, space="PSUM")
            psA = psA_cm.__enter__()
            psc1 = psA.tile([128, 2, ROWS_E], F32, name="psc1")
            psn2 = psA.tile([128, ROWS_E], F32, name="psn2")

            # ---------------- cur1 ----------------
            for fc in range(2):
                for kc in range(4):
                    nc.tensor.matmul(
                        psc1[:, fc, :],
                        lhsT=w1t[:, kc, fc * 128:(fc + 1) * 128],
                        rhs=xt[:, kc, :],
                        start=(kc == 0),
                        stop=False,
                        skip_group_check=True,
                    )
                nc.tensor.matmul(
                    psc1[:, fc, :],
                    lhsT=b1_sb[0:1, fc * 128:(fc + 1) * 128],
                    rhs=ones_row[0:1, 0:ROWS_E],
                    start=False,
                    stop=(fc == 1),
                    skip_group_check=True,
                )
            nc.scalar.dma_start(out=thrp[:], in_=thr_d.ap())
            nc.scalar.dma_start(out=w2t[:], in_=w2t_d.ap())
            nc.scalar.activation(cb1[:], psc1[:], AF.Copy)

            # P1_t = gam_t*cb1 - th_t (ACT), w2t8_t = th_{t+1}*w2t (ACT)
            nc.scalar.activation(
                w2t8[:, 0], w2t[:], AF.Copy, scale=float(TH[2]),
            )

            def gen_p1(t):
                nc.scalar.activation(
                    P1[:, t - 1], cb1[:], AF.Copy,
                    bias=float(-TH[t]), scale=float(GAM[t]),
                )
                nc.scalar.activation(
                    w2t8[:, t - 1], w2t[:], AF.Copy,
                    scale=float(TH[t + 1]),
                )

            for t in range(2, 9):
                gen_p1(t)

            # ---------------- encoder scan + staggered decoder ----------
            def enc_step(t):
                b = bin1[:, t - 1]
                if t == 1:
                    # S1 == 0: bin = (gam_1*cb1 - th_1 > 0) <=> psc1 > 1
                    nc.vector.tensor_scalar(
                        b[:], psc1[:], 1.0, None, AL.is_gt,
                    )
                else:
                    nc.vector.tensor_tensor(
                        out=b[:], in0=P1[:, t - 1], in1=S1[:],
                        op=AL.is_gt,
                    )
                if t < 8:
                    nc.vector.scalar_tensor_tensor(
                        out=S1[:], in0=b[:], scalar=float(TH[t + 1]),
                        in1=S1[:], op0=AL.mult, op1=AL.add,
                    )
                for kc in range(2):
                    nc.tensor.matmul(
                        psn2[:],
                        lhsT=w2t8[:, t - 1, kc, :],
                        rhs=b[:, kc, :],
                        start=(t == 1 and kc == 0),
                        stop=(t == 8 and kc == 1),
                        skip_group_check=True,
                    )
                nc.vector.scalar_tensor_tensor(
                    out=spk01[:, t - 1, :], in0=psn2[:],
                    scalar=thrp[:, t - 1:t], in1=S2[:],
                    op0=AL.subtract, op1=AL.is_gt,
                )
                if t < 8:
                    nc.vector.scalar_tensor_tensor(
                        out=S2[:], in0=spk01[:, t - 1, :],
                        scalar=float(TH[t + 1]), in1=S2[:],
                        op0=AL.mult, op1=AL.add,
                    )
                # incremental cur3: column slice for se = t-1 (fills PE gaps)
                h, sl = (t - 1) // 4, (t - 1) % 4
                for mc in range(2):
                    nc.tensor.matmul(
                        psc3[h][:, mc, sl * 128:(sl + 1) * 128],
                        lhsT=w3t[:, mc, :],
                        rhs=spk01[:, t - 1, :],
                        start=True,
                        stop=True,
                        skip_group_check=True,
                    )

            def half_ramp(h):
                """step-1 spikes + cb3 evac from psc3[h] (already filled)."""
                nc.scalar.activation(cb3[h][0][:], psc3[h][:, 0, :], AF.Copy)
                nc.scalar.activation(cb3[h][1][:], psc3[h][:, 1, :], AF.Copy)
                for mc in range(2):
                    s = sp.tile([128, ROWS_H], F32R, name=f"sneg_{h}_0_{mc}")
                    sneg[h][0][mc] = s
                    nc.vector.tensor_scalar(
                        s[:], psc3[h][:, mc, :],
                        thrp[:, 24 + mc:25 + mc], float(-TH[2]),
                        AL.is_gt, AL.mult,
                    )

            def dec_step(h, t):
                """One decoder step for half h: spikes (t>=2), mm, evac, DMA."""
                if t >= 2:
                    for mc in range(2):
                        s = sp.tile([128, ROWS_H], F32R, name=f"sneg_{h}_{t - 1}_{mc}")
                        sneg[h][t - 1][mc] = s
                        hist = (sneg[h][0][mc][:].bitcast(F32) if t == 2
                                else S3n[h][mc][:])
                        nc.vector.scalar_tensor_tensor(
                            out=n3[h][mc][:], in0=cb3[h][mc][:],
                            scalar=float(GAM[t]), in1=hist,
                            op0=AL.mult, op1=AL.add,
                        )
                        eng = nc.vector if mc == 0 else nc.gpsimd
                        eng.tensor_scalar(
                            s[:], n3[h][mc][:],
                            thrp[:, 8 + 8 * mc + t - 1:9 + 8 * mc + t - 1],
                            float(-TH[t + 1]), AL.is_gt, AL.mult,
                        )
                s = sneg[h][t - 1]
                for kc in range(2):
                    for rc in range(4):
                        nc.tensor.matmul(
                            ps4[h][:, rc, :],
                            lhsT=s[kc][:, rc * 128:(rc + 1) * 128],
                            rhs=w4tn[:, kc, :],
                            start=(t == 1 and kc == 0),
                            stop=(t == 8 and kc == 1),
                            skip_group_check=True,
                        )
                # evacuate this step's snapshot to fp16 and DMA out (ACT only)
                m4 = m4p.tile([128, 4, F4], F16, name=f"m4_{h}")
                dview = out16_d.ap()[t - 1].rearrange("(s p) f -> p s f", p=128)
                if t == 8:
                    # final step: DVE is drained, split evac ACT || DVE
                    nc.scalar.activation(m4[:, 0:3, :], ps4[h][:, 0:3, :], AF.Copy)
                    nc.vector.tensor_copy(out=m4[:, 3:4, :], in_=ps4[h][:, 3:4, :])
                    nc.sync.dma_start(
                        out=dview[:, 4 * h:4 * h + 3, :], in_=m4[:, 0:3, :])
                    nc.sync.dma_start(
                        out=dview[:, 4 * h + 3:4 * h + 4, :], in_=m4[:, 3:4, :])
                else:
                    nc.scalar.activation(m4[:], ps4[h][:], AF.Copy)
                    nc.sync.dma_start(
                        out=dview[:, 4 * h:4 * h + 4, :], in_=m4[:],
                    )
                # spike-history update (off the critical path)
                if t == 2:
                    for mc in range(2):
                        eng = nc.vector if mc == 0 else nc.gpsimd
                        eng.tensor_tensor(
                            out=S3n[h][mc][:],
                            in0=sneg[h][0][mc][:].bitcast(F32),
                            in1=sneg[h][1][mc][:].bitcast(F32), op=AL.add,
                        )
                elif 2 < t < 8:
                    for mc in range(2):
                        eng = nc.vector if mc == 0 else nc.gpsimd
                        eng.tensor_tensor(
                            out=S3n[h][mc][:], in0=S3n[h][mc][:],
                            in1=sneg[h][t - 1][mc][:].bitcast(F32), op=AL.add,
                        )

            # encoder steps 1-8 (decoder-free)
            enc_step(1)
            nc.scalar.dma_start(out=w3t[:], in_=w3t_d.ap())
            nc.gpsimd.dma_start(out=w4tn[:], in_=w4tn_d.ap())
            for t in range(2, 9):
                enc_step(t)
            psA_cm.__exit__(None, None, None)
            # both ramps up front: ACT copies first, then DVE spikes
            for h in range(2):
                nc.scalar.activation(cb3[h][0][:], psc3[h][:, 0, :], AF.Copy)
                nc.scalar.activation(cb3[h][1][:], psc3[h][:, 1, :], AF.Copy)
            for h in range(2):
                for mc in range(2):
                    s = sp.tile([128, ROWS_H], F32R, name=f"sneg_{h}_0_{mc}")
                    sneg[h][0][mc] = s
                    nc.vector.tensor_scalar(
                        s[:], psc3[h][:, mc, :],
                        thrp[:, 24 + mc:25 + mc], float(-TH[2]),
                        AL.is_gt, AL.mult,
                    )
            psc3A_cm.__exit__(None, None, None)
            ps4_cm[0] = tc.tile_pool(name="ps4A", bufs=1, space="PSUM",
                                     side="left")
            ps4[0] = ps4_cm[0].__enter__().tile(
                [128, 4, F4], F32, name="ps4_0")
            psc3B_cm.__exit__(None, None, None)
            ps4_cm[1] = tc.tile_pool(name="ps4B", bufs=1, space="PSUM",
                                     side="right")
            ps4[1] = ps4_cm[1].__enter__().tile(
                [128, 4, F4], F32, name="ps4_1")
            for t in range(1, 9):
                dec_step(0, t)
                dec_step(1, t)
            ps4_cm[1].__exit__(None, None, None)
            ps4_cm[0].__exit__(None, None, None)

    nc.compile()
    return nc


_NC_CACHE = None


def _get_module():
    global _NC_CACHE
    if _NC_CACHE is None:
        _NC_CACHE = build_module()
    return _NC_CACHE


def _prep_shared(W1, b1, W2, b2, W3, b3, W4):
    f = np.float32
    w1t = np.ascontiguousarray(W1.T.reshape(4, 128, H1).transpose(1, 0, 2), f)
    b1r = np.ascontiguousarray(b1.reshape(1, H1), f)
    w2t = np.ascontiguousarray(
        (BETA * W2.T).astype(f).reshape(2, 128, H2).transpose(1, 0, 2), f)
    w3t = np.ascontiguousarray(
        W3.T.reshape(128, 2, H2), f)  # [in128, mc, out128]: W3.T is [128, 256]
    w4tn = np.ascontiguousarray(
        (-BETA * W4.T).astype(f).reshape(2, 128, F4).transpose(1, 0, 2), f)

    thrp = np.zeros((128, 26), f)
    for t in range(1, 9):
        thrp[:, t - 1] = TH[t] - GAM[t] * b2
    thr3 = np.stack([(TH[t] - GAM[t] * b3).astype(f) for t in range(1, 9)])  # [8,256]
    for mc in range(2):
        thrp[:, 8 + 8 * mc:16 + 8 * mc] = thr3[:, mc * 128:(mc + 1) * 128].T
        thrp[:, 24 + mc] = (BETA * thr3[0, mc * 128:(mc + 1) * 128]).astype(f)
    return dict(
        w1t=w1t, b1r=b1r, w2t=w2t, w3t=w3t, w4tn=w4tn,
        thrp=np.ascontiguousarray(thrp),
    )


def kernel(x, W1, b1, W2, b2, W3, b3, W4, b4):
    f = np.float32
    x = np.asarray(x, f)
    shared = _prep_shared(
        np.asarray(W1, f), np.asarray(b1, f), np.asarray(W2, f),
        np.asarray(b2, f), np.asarray(W3, f), np.asarray(b3, f),
        np.asarray(W4, f))
    b4 = np.asarray(b4, f)

    nc = _get_module()
    in_maps = []
    for i in range(NCORES):
        m = dict(shared)
        xs = x[:, i * BS:(i + 1) * BS, :].reshape(ROWS_E, F_IN)  # rows (t,b)
        m["xt"] = np.ascontiguousarray(
            xs.T.reshape(4, 128, ROWS_E).transpose(1, 0, 2), f)
        in_maps.append(m)

    trace = os.environ.get("KERNEL_TRACE", "0") == "1"
    res = run_bass_kernel_spmd(
        nc, in_maps, core_ids=list(range(NCORES)), trace=trace
    )
    if trace and res.exec_time_ns is not None:
        print(f"HW exec time: {res.exec_time_ns} ns")

    # host reconstruction: mem[t] = N4_t * beta^t + (beta^t*gam_t) * b4
    scale = np.array([BPOW[t] for t in range(1, 9)], f)
    bias = scale[:, None] * np.array([GAM[t] for t in range(1, 9)], f)[:, None] \
        * b4[None, :]
    mem = np.empty((T, T, T, B, F4), dtype=f)
    for i in range(NCORES):
        n4 = res.results[i]["out16"].astype(f)  # [8, 1024(se,t,b), 512]
        m4 = n4 * scale[:, None, None] + bias[:, None, :]
        mem[:, :, :, i * BS:(i + 1) * BS, :] = m4.reshape(T, T, T, BS, F4)
    spk = np.zeros((T, T, T, B, F4), dtype=f)
    return mem, spk
